# revision 8
# baseline (speedup 1.0000x reference)
"""NT-Xent contrastive loss on 8 Trainium2 NeuronCores.

Math (reference): z = l2-normalize rows of concat(emb_i, emb_j) -> [8192, 512].
sim = (z @ z.T) / T with T = 0.5.  denom_r = sum_j exp(sim_rj) - exp(sim_rr),
sim_rr = 1/T exactly, so subtract e^2.  pos pair sim[k, k+N] = 2*cos_k.
loss = (sum_r log(denom_r) - 4 * sum_k cos_k) / 8192.

Sharding: data-parallel over rows of sim.  Each core computes a 1024-row
block of sim against all 8192 columns, reduces to one partial scalar, plus
a 512-pair slice of the positive-pair cosines.  Host sums the 8 partials.

Device pipeline per core (identical SPMD program, per-core data):
  - stream repsT [512, 8192] f32 (host-transposed) in [128, 512] tiles
  - column sums of squares: ACT Square -> PE ones-matmul (contract over
    the 4 partition chunks), batched in groups of 4 column tiles
  - per batch: rinv = exp(-0.5*ln(ss)) on ACT (ln/exp share one table
    set with square -> no ACT table thrashing anywhere in the kernel)
  - column scale + bf16 cast in one DVE pass: zT = st_f32 * bcast(rinv)
    (partition_broadcast on GPSIMD makes the [128, 512] scale tile)
  - main matmul: lhsT = own 1024 normalized cols, rhs = all 8192 cols,
    K=512 over 4 chunks, PSUM groups of [128, 2048]
  - ACT exp(2*x) with accum_out -> row sums, ln(denom - e^2), reduce
  - batches are pipelined: the first matmul group starts after ~2 of 5
    prep batches, so PE ramps at ~20us instead of waiting for full prep
"""

import math

import numpy as np

import concourse.bacc as bacc
import concourse.bass as bass
import concourse.tile as tile
from concourse import mybir
from concourse.bass_utils import run_bass_kernel_spmd

F32 = mybir.dt.float32
BF16 = mybir.dt.bfloat16
AF = mybir.ActivationFunctionType
ALU = mybir.AluOpType

N_CORES = 8
N = 4096              # rows per input
D = 512               # embedding dim
M = 2 * N             # 8192 rows of sim
ROWS_PER_CORE = M // N_CORES      # 1024
POS_PER_CORE = N // N_CORES       # 512
D_CH = D // 128       # 4 contraction chunks
NJT = M // 512        # 16 column tiles of 512
MYJT = ROWS_PER_CORE // 512       # 2
E2 = float(math.exp(2.0))
INV_T = 2.0           # 1 / temperature


def build_program():
    nc = bacc.Bacc(
        "TRN2",
        target_bir_lowering=False,
        debug=False,
        num_devices=N_CORES,
    )

    repsT = nc.dram_tensor("repsT", [D, M], F32, kind="ExternalInput")
    myT = nc.dram_tensor("myT", [D, ROWS_PER_CORE], F32, kind="ExternalInput")
    pi = nc.dram_tensor("pi", [POS_PER_CORE, D], F32, kind="ExternalInput")
    pj = nc.dram_tensor("pj", [POS_PER_CORE, D], F32, kind="ExternalInput")
    out_d = nc.dram_tensor("out", [2, 1], F32, kind="ExternalOutput")

    with tile.TileContext(nc) as tc:
        import contextlib

        with contextlib.ExitStack() as ctx:
            const = ctx.enter_context(tc.tile_pool(name="const", bufs=1))
            big = ctx.enter_context(tc.tile_pool(name="big", bufs=1))
            stage = ctx.enter_context(tc.tile_pool(name="stage", bufs=22))
            sqp = ctx.enter_context(tc.tile_pool(name="sqp", bufs=8))
            bpool = ctx.enter_context(tc.tile_pool(name="bpool", bufs=4))
            rowp = ctx.enter_context(tc.tile_pool(name="rowp", bufs=2))
            sink = ctx.enter_context(tc.tile_pool(name="sink", bufs=2))
            esink = ctx.enter_context(tc.tile_pool(name="esink", bufs=2))

            ones_bf = const.tile([128, 1], BF16)
            nc.vector.memset(ones_bf[:], 1.0)
            ones_f = const.tile([128, 1], F32)
            nc.vector.memset(ones_f[:], 1.0)
            neg_e2 = const.tile([128, 1], F32)
            nc.vector.memset(neg_e2[:], -E2)

            # persistent tensors
            zT = [big.tile([128, M], BF16, tag=f"zT{d}", name=f"zT{d}")
                  for d in range(D_CH)]
            lhsT = [big.tile([128, ROWS_PER_CORE], BF16, tag=f"lhsT{d}",
                             name=f"lhsT{d}") for d in range(D_CH)]
            dacc = big.tile([128, 32], F32, tag="dacc")
            pos_ssi = big.tile([128, 4], F32, tag="pos_ssi")
            pos_ssj = big.tile([128, 4], F32, tag="pos_ssj")
            pos_dot = big.tile([128, 4], F32, tag="pos_dot")

            # ---------------- Phase A: positive-pair cosines ----------------
            for t in range(4):
                pit = stage.tile([128, D], F32, tag="stage")
                nc.sync.dma_start(pit[:], pi[bass.ts(t, 128), :])
                pjt = stage.tile([128, D], F32, tag="stage")
                nc.sync.dma_start(pjt[:], pj[bass.ts(t, 128), :])
                for src0, src1, acc in (
                    (pit, pit, pos_ssi),
                    (pjt, pjt, pos_ssj),
                    (pit, pjt, pos_dot),
                ):
                    snk = sink.tile([128, D], F32, tag="sink")
                    nc.vector.tensor_mul(snk[:], src0[:], src1[:])
                    nc.vector.tensor_reduce(
                        acc[:, t : t + 1], snk[:],
                        axis=mybir.AxisListType.X, op=ALU.add,
                    )
            lssi = big.tile([128, 4], F32, tag="lssi")
            lssj = big.tile([128, 4], F32, tag="lssj")
            nc.scalar.activation(lssi[:], pos_ssi[:], AF.Ln)
            nc.scalar.activation(lssj[:], pos_ssj[:], AF.Ln)
            lsum = big.tile([128, 4], F32, tag="lsum")
            nc.vector.tensor_add(lsum[:], lssi[:], lssj[:])
            rinv_ij = big.tile([128, 4], F32, tag="rinv_ij")
            nc.scalar.activation(rinv_ij[:], lsum[:], AF.Exp, scale=-0.5)
            posk = big.tile([128, 4], F32, tag="posk")
            nc.vector.tensor_mul(posk[:], pos_dot[:], rinv_ij[:])

            # ------------- Phase B: normalized transposed reps, batched -----
            # batch = list of (src_dram, src_coltile, dst_tiles, dst_coltile)
            batches = []
            batches.append([(myT, j, lhsT, j) for j in range(MYJT)])
            for g in range(4):
                batches.append([(repsT, 4 * g + j, zT, 4 * g + j)
                                for j in range(4)])

            with tc.tile_pool(name="pp_ss", bufs=4, space="PSUM") as pp_ss:
                for bi, batch in enumerate(batches):
                    nb = len(batch)
                    ss_b = rowp.tile([1, 512 * nb], F32, tag="rowp",
                                     name=f"ss_b{bi}")
                    sts = []
                    for k, (src, sj, dst, dj) in enumerate(batch):
                        pst = pp_ss.tile([1, 512], F32, tag="pp_ss")
                        st4 = []
                        for d in range(D_CH):
                            st = stage.tile([128, 512], F32, tag="stage")
                            nc.sync.dma_start(
                                st[:], src[bass.ts(d, 128), bass.ts(sj, 512)]
                            )
                            st4.append(st)
                            sqt = sqp.tile([128, 512], BF16, tag="sqp")
                            nc.scalar.activation(sqt[:], st[:], AF.Square)
                            nc.tensor.matmul(
                                pst[:], ones_bf[:], sqt[:],
                                start=(d == 0), stop=(d == D_CH - 1),
                            )
                        sts.append(st4)
                        nc.vector.tensor_copy(
                            ss_b[0:1, bass.ts(k, 512)], pst[:]
                        )
                    # batch barrier: rinv = exp(-0.5 ln(ss)) on one ACT lane
                    lss_b = rowp.tile([1, 512 * nb], F32, tag="rowp2",
                                      name=f"lss_b{bi}")
                    nc.scalar.activation(lss_b[:], ss_b[:], AF.Ln)
                    rinv_b = rowp.tile([1, 512 * nb], F32, tag="rowp3",
                                       name=f"rinv_b{bi}")
                    nc.scalar.activation(rinv_b[:], lss_b[:], AF.Exp,
                                         scale=-0.5)
                    for k, (src, sj, dst, dj) in enumerate(batch):
                        bt = bpool.tile([128, 512], F32, tag="bpool")
                        nc.gpsimd.partition_broadcast(
                            bt[:], rinv_b[0:1, bass.ts(k, 512)]
                        )
                        for d in range(D_CH):
                            nc.vector.tensor_mul(
                                dst[d][:, bass.ts(dj, 512)],
                                sts[k][d][:], bt[:],
                            )

            # ---------------- Phase C: sim block, exp, row sums --------------
            with tc.tile_pool(name="pp_main", bufs=2, space="PSUM") as pp_main:
                for jg in range(4):
                    for i in range(8):
                        pt = pp_main.tile([128, 2048], F32, tag="pp_main")
                        for d in range(D_CH):
                            for jj in range(4):
                                j = jg * 4 + jj
                                nc.tensor.matmul(
                                    pt[:, bass.ts(jj, 512)],
                                    lhsT[d][:, bass.ts(i, 128)],
                                    zT[d][:, bass.ts(j, 512)],
                                    start=(d == 0), stop=(d == D_CH - 1),
                                )
                        es = esink.tile([128, 2048], BF16, tag="esink")
                        k = i * 4 + jg
                        nc.scalar.activation(
                            es[:], pt[:], AF.Exp, scale=INV_T,
                            accum_out=dacc[:, k : k + 1],
                        )

            # ---------------- Final reduction --------------------------------
            dn = big.tile([128, 8], F32, tag="dn")
            nc.vector.tensor_reduce(
                dn[:], dacc[:].rearrange("p (i g) -> p i g", g=4),
                axis=mybir.AxisListType.X, op=ALU.add,
            )
            ld = big.tile([128, 8], F32, tag="ld")
            nc.scalar.activation(ld[:], dn[:], AF.Ln, bias=neg_e2[:])
            fin = big.tile([128, 2], F32, tag="fin")
            nc.vector.tensor_reduce(
                fin[:, 0:1], ld[:], axis=mybir.AxisListType.X, op=ALU.add
            )
            nc.vector.tensor_reduce(
                fin[:, 1:2], posk[:], axis=mybir.AxisListType.X, op=ALU.add
            )
            with tc.tile_pool(name="pp_fin", bufs=1, space="PSUM") as pp_fin:
                fmm = pp_fin.tile([2, 1], F32, tag="pp_fin")
                nc.tensor.matmul(fmm[:], fin[:], ones_f[:], start=True,
                                 stop=True)
                outsb = big.tile([2, 1], F32, tag="outsb")
                nc.vector.tensor_copy(outsb[:], fmm[:])
            nc.sync.dma_start(out_d[:], outsb[:])

    nc.compile()
    return nc


_NC_CACHE = None


def _get_program():
    global _NC_CACHE
    if _NC_CACHE is None:
        _NC_CACHE = build_program()
    return _NC_CACHE


def make_in_maps(emb_i: np.ndarray, emb_j: np.ndarray):
    emb_i = np.asarray(emb_i, dtype=np.float32)
    emb_j = np.asarray(emb_j, dtype=np.float32)
    reps = np.concatenate([emb_i, emb_j], axis=0)          # [8192, 512]
    repsT = np.ascontiguousarray(reps.T)                   # [512, 8192]
    in_maps = []
    for c in range(N_CORES):
        in_maps.append(
            {
                "repsT": repsT,
                "myT": np.ascontiguousarray(
                    repsT[:, c * ROWS_PER_CORE : (c + 1) * ROWS_PER_CORE]
                ),
                "pi": np.ascontiguousarray(
                    emb_i[c * POS_PER_CORE : (c + 1) * POS_PER_CORE]
                ),
                "pj": np.ascontiguousarray(
                    emb_j[c * POS_PER_CORE : (c + 1) * POS_PER_CORE]
                ),
            }
        )
    return in_maps


def combine_outputs(results):
    ld_sum = 0.0
    cos_sum = 0.0
    for r in results:
        o = np.asarray(r["out"], dtype=np.float64).reshape(-1)
        ld_sum += o[0]
        cos_sum += o[1]
    loss = (ld_sum - 2.0 * INV_T * cos_sum) / float(M)
    return np.float32(loss)


def kernel(emb_i: np.ndarray, emb_j: np.ndarray) -> np.ndarray:
    nc = _get_program()
    in_maps = make_in_maps(emb_i, emb_j)
    res = run_bass_kernel_spmd(nc, in_maps, list(range(N_CORES)))
    return combine_outputs(res.results)


# revision 10
# speedup vs baseline: 1.0574x; 1.0574x over previous
"""NT-Xent contrastive loss on 8 Trainium2 NeuronCores.

Math (reference): z = l2-normalize rows of concat(emb_i, emb_j) -> [8192, 512].
sim = (z @ z.T) / T with T = 0.5.  denom_r = sum_j exp(sim_rj) - exp(sim_rr),
sim_rr = 1/T exactly, so subtract e^2.  pos pair sim[k, k+N] = 2*cos_k.
loss = (sum_r log(denom_r) - 4 * sum_k cos_k) / 8192.

Sharding: data-parallel over rows of sim.  Each core computes a 1024-row
block of sim against all 8192 columns, reduces to one partial scalar, plus
a 512-pair slice of the positive-pair cosines.  Host sums the 8 partials.

Device pipeline per core (identical SPMD program, per-core data):
  - stream repsT [512, 8192] f32 (host-transposed) in [128, 512] tiles
  - column sums of squares via ones[128,128]-matmul of squares: the PSUM
    result is REPLICATED across all 128 partitions, so rinv =
    exp(-0.5*ln(ss)) runs at full 128-lane ACT rate straight out of PSUM
    and the Exp output IS the [128, 512] per-column scale tile (no
    partition broadcast, no 1-lane row ops)
  - column scale + bf16 cast in one DVE pass: zT = st_f32 * B_f32 -> bf16
  - all ACT functions used (Square/Ln/Exp/Copy) live in the single
    natural_log_exp_and_others table set; the table chooser is pinned to
    it, so exactly one ACT_TABLE_LOAD in the whole kernel
  - main matmul: lhsT = own 1024 normalized cols, rhs = all 8192 cols,
    K=512 over 4 chunks, PSUM groups [128, 2048], bf16
  - ACT exp(2*x) with accum_out -> row sums, ln(denom - e^2), reduce
  - emission interleaves prep of column-group g with matmuls of group
    g-1 so no engine FIFO ever blocks the pipeline
"""

import functools
import math

import numpy as np

import concourse.bacc as bacc
import concourse.bass as bass
import concourse.tile as tile
from concourse import mybir
from concourse.bass_utils import run_bass_kernel_spmd
from concourse.hw_specs import get_activation_tables as _orig_gat

F32 = mybir.dt.float32
BF16 = mybir.dt.bfloat16
AF = mybir.ActivationFunctionType
ALU = mybir.AluOpType

N_CORES = 8
N = 4096              # rows per input
D = 512               # embedding dim
M = 2 * N             # 8192 rows of sim
ROWS_PER_CORE = M // N_CORES      # 1024
POS_PER_CORE = N // N_CORES       # 512
D_CH = D // 128       # 4 contraction chunks
NJT = M // 512        # 16 column tiles of 512
MYJT = ROWS_PER_CORE // 512       # 2
E2 = float(math.exp(2.0))
INV_T = 2.0           # 1 / temperature

_ONE_SET = "natural_log_exp_and_others"


@functools.cache
def _patched_gat(arch):
    """Pin every ACT function this kernel uses to one table set so the
    table-load chooser emits a single ACT_TABLE_LOAD (the default
    first-match policy alternates sets on every Ln<->Exp transition,
    costing ~2.7us per switch)."""
    t = dict(_orig_gat(arch))
    if _ONE_SET not in t:
        return t
    mine = {AF.Exp, AF.Ln, AF.Square, AF.Copy, AF.Identity}
    return {
        name: (s if name == _ONE_SET else (set(s) - mine))
        for name, s in t.items()
    }


def build_program():
    bacc.get_activation_tables = _patched_gat

    nc = bacc.Bacc(
        "TRN2",
        target_bir_lowering=False,
        debug=False,
        num_devices=N_CORES,
    )

    repsT = nc.dram_tensor("repsT", [D, M], F32, kind="ExternalInput")
    myT = nc.dram_tensor("myT", [D, ROWS_PER_CORE], F32, kind="ExternalInput")
    pi = nc.dram_tensor("pi", [POS_PER_CORE, D], F32, kind="ExternalInput")
    pj = nc.dram_tensor("pj", [POS_PER_CORE, D], F32, kind="ExternalInput")
    out_d = nc.dram_tensor("out", [2, 1], F32, kind="ExternalOutput")

    with tile.TileContext(nc) as tc:
        import contextlib

        with contextlib.ExitStack() as ctx:
            const = ctx.enter_context(tc.tile_pool(name="const", bufs=1))
            big = ctx.enter_context(tc.tile_pool(name="big", bufs=1))
            stage = ctx.enter_context(tc.tile_pool(name="stage", bufs=22))
            sqp = ctx.enter_context(tc.tile_pool(name="sqp", bufs=8))
            bpool = ctx.enter_context(tc.tile_pool(name="bpool", bufs=6))
            lnp = ctx.enter_context(tc.tile_pool(name="lnp", bufs=4))
            sink = ctx.enter_context(tc.tile_pool(name="sink", bufs=2))
            esink = ctx.enter_context(tc.tile_pool(name="esink", bufs=2))

            ones128 = const.tile([128, 128], BF16)
            nc.vector.memset(ones128[:], 1.0)
            ones_f = const.tile([128, 1], F32)
            nc.vector.memset(ones_f[:], 1.0)
            neg_e2 = const.tile([128, 1], F32)
            nc.vector.memset(neg_e2[:], -E2)

            zT = [big.tile([128, M], BF16, tag=f"zT{d}", name=f"zT{d}")
                  for d in range(D_CH)]
            lhsT = [big.tile([128, ROWS_PER_CORE], BF16, tag=f"lhsT{d}",
                             name=f"lhsT{d}") for d in range(D_CH)]
            dacc = big.tile([128, 32], F32, tag="dacc")
            pos_ssi = big.tile([128, 4], F32, tag="pos_ssi")
            pos_ssj = big.tile([128, 4], F32, tag="pos_ssj")
            pos_dot = big.tile([128, 4], F32, tag="pos_dot")

            pp_main = ctx.enter_context(
                tc.tile_pool(name="pp_main", bufs=2, space="PSUM")
            )

            def emit_prep_group(src, src_j0, dst, dst_j0, njt):
                """Normalize njt (<=4) column tiles of 512: squares ->
                replicated column sum-of-squares (PSUM) -> rinv tile ->
                scale+cast into dst."""
                pt = pp_main.tile([128, 2048], F32, tag="pp_main",
                                  name=f"ssg_{src.name}_{src_j0}")
                sts = []
                for k in range(njt):
                    st4 = []
                    for d in range(D_CH):
                        st = stage.tile([128, 512], F32, tag="stage")
                        nc.sync.dma_start(
                            st[:],
                            src[bass.ts(d, 128), bass.ts(src_j0 + k, 512)],
                        )
                        st4.append(st)
                        sqt = sqp.tile([128, 512], BF16, tag="sqp")
                        if d < 2:
                            nc.scalar.activation(sqt[:], st[:], AF.Square)
                        else:
                            nc.vector.tensor_mul(sqt[:], st[:], st[:])
                        nc.tensor.matmul(
                            pt[:, bass.ts(k, 512)], ones128[:], sqt[:],
                            start=(d == 0), stop=(d == D_CH - 1),
                        )
                    sts.append(st4)
                bts = []
                for k in range(njt):
                    lt = lnp.tile([128, 512], F32, tag="lnp")
                    nc.scalar.activation(lt[:], pt[:, bass.ts(k, 512)], AF.Ln)
                    bts.append(lt)
                for k in range(njt):
                    bt = bpool.tile([128, 512], F32, tag="bpool")
                    nc.scalar.activation(bt[:], bts[k][:], AF.Exp, scale=-0.5)
                    bts[k] = bt
                for k in range(njt):
                    for d in range(D_CH):
                        nc.vector.tensor_mul(
                            dst[d][:, bass.ts(dst_j0 + k, 512)],
                            sts[k][d][:], bts[k][:],
                        )

            # ------- lhsT: own 1024 columns, normalized ----------------------
            emit_prep_group(myT, 0, lhsT, 0, MYJT)

            # ------- positive-pair cosines ----------------------------------
            for t in range(4):
                pit = stage.tile([128, D], F32, tag="stage")
                nc.sync.dma_start(pit[:], pi[bass.ts(t, 128), :])
                pjt = stage.tile([128, D], F32, tag="stage")
                nc.sync.dma_start(pjt[:], pj[bass.ts(t, 128), :])
                for src0, src1, acc in (
                    (pit, pit, pos_ssi),
                    (pjt, pjt, pos_ssj),
                    (pit, pjt, pos_dot),
                ):
                    snk = sink.tile([128, D], F32, tag="sink")
                    nc.vector.tensor_mul(snk[:], src0[:], src1[:])
                    nc.vector.tensor_reduce(
                        acc[:, t : t + 1], snk[:],
                        axis=mybir.AxisListType.X, op=ALU.add,
                    )
            lssi = big.tile([128, 4], F32, tag="lssi")
            lssj = big.tile([128, 4], F32, tag="lssj")
            nc.scalar.activation(lssi[:], pos_ssi[:], AF.Ln)
            nc.scalar.activation(lssj[:], pos_ssj[:], AF.Ln)
            lsum = big.tile([128, 4], F32, tag="lsum")
            nc.vector.tensor_add(lsum[:], lssi[:], lssj[:])
            rinv_ij = big.tile([128, 4], F32, tag="rinv_ij")
            nc.scalar.activation(rinv_ij[:], lsum[:], AF.Exp, scale=-0.5)
            posk = big.tile([128, 4], F32, tag="posk")
            nc.vector.tensor_mul(posk[:], pos_dot[:], rinv_ij[:])

            # ------- main: per j-group, prep 4 column tiles then matmul ------
            for jg in range(4):
                emit_prep_group(repsT, 4 * jg, zT, 4 * jg, 4)
                for i in range(8):
                    pt = pp_main.tile([128, 2048], F32, tag="pp_main",
                                      name=f"mm_{jg}_{i}")
                    for d in range(D_CH):
                        for jj in range(4):
                            j = jg * 4 + jj
                            nc.tensor.matmul(
                                pt[:, bass.ts(jj, 512)],
                                lhsT[d][:, bass.ts(i, 128)],
                                zT[d][:, bass.ts(j, 512)],
                                start=(d == 0), stop=(d == D_CH - 1),
                            )
                    es = esink.tile([128, 2048], BF16, tag="esink")
                    k = i * 4 + jg
                    nc.scalar.activation(
                        es[:], pt[:], AF.Exp, scale=INV_T,
                        accum_out=dacc[:, k : k + 1],
                    )

            # ------- final reduction ----------------------------------------
            dn = big.tile([128, 8], F32, tag="dn")
            nc.vector.tensor_reduce(
                dn[:], dacc[:].rearrange("p (i g) -> p i g", g=4),
                axis=mybir.AxisListType.X, op=ALU.add,
            )
            ld = big.tile([128, 8], F32, tag="ld")
            nc.scalar.activation(ld[:], dn[:], AF.Ln, bias=neg_e2[:])
            fin = big.tile([128, 2], F32, tag="fin")
            nc.vector.tensor_reduce(
                fin[:, 0:1], ld[:], axis=mybir.AxisListType.X, op=ALU.add
            )
            nc.vector.tensor_reduce(
                fin[:, 1:2], posk[:], axis=mybir.AxisListType.X, op=ALU.add
            )
            fmm = pp_main.tile([128, 2048], F32, tag="pp_main", name="fmm")
            nc.tensor.matmul(fmm[0:2, 0:1], fin[:], ones_f[:], start=True,
                             stop=True)
            outsb = big.tile([2, 1], F32, tag="outsb")
            nc.vector.tensor_copy(outsb[:], fmm[0:2, 0:1])
            nc.sync.dma_start(out_d[:], outsb[:])

    nc.compile()
    return nc


_NC_CACHE = None


def _get_program():
    global _NC_CACHE
    if _NC_CACHE is None:
        _NC_CACHE = build_program()
    return _NC_CACHE


def make_in_maps(emb_i: np.ndarray, emb_j: np.ndarray):
    emb_i = np.asarray(emb_i, dtype=np.float32)
    emb_j = np.asarray(emb_j, dtype=np.float32)
    reps = np.concatenate([emb_i, emb_j], axis=0)          # [8192, 512]
    repsT = np.ascontiguousarray(reps.T)                   # [512, 8192]
    in_maps = []
    for c in range(N_CORES):
        in_maps.append(
            {
                "repsT": repsT,
                "myT": np.ascontiguousarray(
                    repsT[:, c * ROWS_PER_CORE : (c + 1) * ROWS_PER_CORE]
                ),
                "pi": np.ascontiguousarray(
                    emb_i[c * POS_PER_CORE : (c + 1) * POS_PER_CORE]
                ),
                "pj": np.ascontiguousarray(
                    emb_j[c * POS_PER_CORE : (c + 1) * POS_PER_CORE]
                ),
            }
        )
    return in_maps


def combine_outputs(results):
    ld_sum = 0.0
    cos_sum = 0.0
    for r in results:
        o = np.asarray(r["out"], dtype=np.float64).reshape(-1)
        ld_sum += o[0]
        cos_sum += o[1]
    loss = (ld_sum - 2.0 * INV_T * cos_sum) / float(M)
    return np.float32(loss)


def kernel(emb_i: np.ndarray, emb_j: np.ndarray) -> np.ndarray:
    nc = _get_program()
    in_maps = make_in_maps(emb_i, emb_j)
    res = run_bass_kernel_spmd(nc, in_maps, list(range(N_CORES)))
    return combine_outputs(res.results)


# revision 13
# speedup vs baseline: 1.2421x; 1.1747x over previous
"""NT-Xent contrastive loss on 8 Trainium2 NeuronCores.

Math (reference): z = l2-normalize rows of concat(emb_i, emb_j) -> [8192, 512].
sim = (z @ z.T) / T with T = 0.5.  denom_r = sum_j exp(sim_rj) - exp(sim_rr),
sim_rr = 1/T exactly, so subtract e^2.  pos pair sim[k, k+N] = 2*cos_k.
loss = (sum_r log(denom_r) - 4 * sum_k cos_k) / 8192.

Sharding: data-parallel over rows of sim.  Each core computes a 1024-row
block of sim against all 8192 columns, reduces to one partial scalar, plus
a 512-pair slice of the positive-pair cosines.  Host sums the 8 partials.

Device pipeline per core (identical SPMD program, per-core data):
  - stream repsT [512, 8192] f32 (host-transposed) in [128, 512] tiles
  - column sums of squares via ones[128,128]-matmul of squares: the PSUM
    result is REPLICATED across all 128 partitions, so rinv =
    exp(-0.5*ln(ss)) runs at full 128-lane ACT rate straight out of PSUM
    and the Exp output IS the [128, 512] per-column scale tile (no
    partition broadcast, no 1-lane row ops)
  - column scale + bf16 cast in one DVE pass: zT = st_f32 * B_f32 -> bf16
  - all ACT functions used (Square/Ln/Exp/Copy) live in the single
    natural_log_exp_and_others table set; the table chooser is pinned to
    it, so exactly one ACT_TABLE_LOAD in the whole kernel
  - main matmul: lhsT = own 1024 normalized cols, rhs = all 8192 cols,
    K=512 over 4 chunks, PSUM groups [128, 2048], bf16
  - ACT exp(2*x) with accum_out -> row sums, ln(denom - e^2), reduce
  - emission interleaves prep of column-group g with matmuls of group
    g-1 so no engine FIFO ever blocks the pipeline
"""

import functools
import math

import numpy as np

import concourse.bacc as bacc
import concourse.bass as bass
import concourse.tile as tile
from concourse import mybir
from concourse.bass_utils import run_bass_kernel_spmd
from concourse.hw_specs import get_activation_tables as _orig_gat

F32 = mybir.dt.float32
BF16 = mybir.dt.bfloat16
AF = mybir.ActivationFunctionType
ALU = mybir.AluOpType

N_CORES = 8
N = 4096              # rows per input
D = 512               # embedding dim
M = 2 * N             # 8192 rows of sim
ROWS_PER_CORE = M // N_CORES      # 1024
POS_PER_CORE = N // N_CORES       # 512
D_CH = D // 128       # 4 contraction chunks
NJT = M // 512        # 16 column tiles of 512
MYJT = ROWS_PER_CORE // 512       # 2
E2 = float(math.exp(2.0))
INV_T = 2.0           # 1 / temperature

_ONE_SET = "natural_log_exp_and_others"


@functools.cache
def _patched_gat(arch):
    """Pin every ACT function this kernel uses to one table set so the
    table-load chooser emits a single ACT_TABLE_LOAD (the default
    first-match policy alternates sets on every Ln<->Exp transition,
    costing ~2.7us per switch)."""
    t = dict(_orig_gat(arch))
    if _ONE_SET not in t:
        return t
    mine = {AF.Exp, AF.Ln, AF.Square, AF.Copy, AF.Identity}
    return {
        name: (s if name == _ONE_SET else (set(s) - mine))
        for name, s in t.items()
    }


def build_program():
    bacc.get_activation_tables = _patched_gat

    nc = bacc.Bacc(
        "TRN2",
        target_bir_lowering=False,
        debug=False,
        num_devices=N_CORES,
    )

    repsT = nc.dram_tensor("repsT", [D, M], F32, kind="ExternalInput")
    myT = nc.dram_tensor("myT", [D, ROWS_PER_CORE], F32, kind="ExternalInput")
    pi = nc.dram_tensor("pi", [POS_PER_CORE, D], F32, kind="ExternalInput")
    pj = nc.dram_tensor("pj", [POS_PER_CORE, D], F32, kind="ExternalInput")
    out_d = nc.dram_tensor("out", [2, 1], F32, kind="ExternalOutput")

    with tile.TileContext(nc) as tc:
        import contextlib

        with contextlib.ExitStack() as ctx:
            const = ctx.enter_context(tc.tile_pool(name="const", bufs=1))
            big = ctx.enter_context(tc.tile_pool(name="big", bufs=1))
            stage = ctx.enter_context(tc.tile_pool(name="stage", bufs=24))
            sqp = ctx.enter_context(tc.tile_pool(name="sqp", bufs=8))
            bpool = ctx.enter_context(tc.tile_pool(name="bpool", bufs=8))
            lnp = ctx.enter_context(tc.tile_pool(name="lnp", bufs=4))
            sink = ctx.enter_context(tc.tile_pool(name="sink", bufs=2))
            esink = ctx.enter_context(tc.tile_pool(name="esink", bufs=2))

            ones128 = const.tile([128, 128], BF16)
            nc.vector.memset(ones128[:], 1.0)
            ones_f = const.tile([128, 1], F32)
            nc.vector.memset(ones_f[:], 1.0)
            neg_e2 = const.tile([128, 1], F32)
            nc.vector.memset(neg_e2[:], -E2)

            zT = [big.tile([128, M], BF16, tag=f"zT{d}", name=f"zT{d}")
                  for d in range(D_CH)]
            lhsT = [big.tile([128, ROWS_PER_CORE], BF16, tag=f"lhsT{d}",
                             name=f"lhsT{d}") for d in range(D_CH)]
            dacc = big.tile([128, 32], F32, tag="dacc")
            pos_ssi = big.tile([128, 4], F32, tag="pos_ssi")
            pos_ssj = big.tile([128, 4], F32, tag="pos_ssj")
            pos_dot = big.tile([128, 4], F32, tag="pos_dot")

            pp_main = ctx.enter_context(
                tc.tile_pool(name="pp_main", bufs=2, space="PSUM")
            )

            def emit_prep_group(src, src_j0, dst, dst_j0, njt):
                """Normalize njt (<=4) column tiles of 512: squares ->
                replicated column sum-of-squares (PSUM) -> rinv tile ->
                scale+cast into dst."""
                pt = pp_main.tile([128, 2048], F32, tag="pp_main",
                                  name=f"ssg_{src.name}_{src_j0}")
                sts = []
                for k in range(njt):
                    st4 = []
                    for d in range(D_CH):
                        st = stage.tile([128, 512], F32, tag="stage")
                        nc.sync.dma_start(
                            st[:],
                            src[bass.ts(d, 128), bass.ts(src_j0 + k, 512)],
                        )
                        st4.append(st)
                        sqt = sqp.tile([128, 512], BF16, tag="sqp")
                        if d < 2:
                            nc.scalar.activation(sqt[:], st[:], AF.Square)
                        else:
                            nc.vector.tensor_mul(sqt[:], st[:], st[:])
                        nc.tensor.matmul(
                            pt[:, bass.ts(k, 512)], ones128[:], sqt[:],
                            start=(d == 0), stop=(d == D_CH - 1),
                        )
                    sts.append(st4)
                bts = []
                for k in range(njt):
                    lt = lnp.tile([128, 512], F32, tag="lnp")
                    nc.scalar.activation(lt[:], pt[:, bass.ts(k, 512)], AF.Ln)
                    bts.append(lt)
                for k in range(njt):
                    bt = bpool.tile([128, 512], F32, tag="bpool")
                    nc.scalar.activation(bt[:], bts[k][:], AF.Exp, scale=-0.5)
                    bts[k] = bt
                for k in range(njt):
                    for d in range(D_CH):
                        nc.vector.tensor_mul(
                            dst[d][:, bass.ts(dst_j0 + k, 512)],
                            sts[k][d][:], bts[k][:],
                        )

            # ------- lhsT: own 1024 columns, normalized ----------------------
            emit_prep_group(myT, 0, lhsT, 0, MYJT)

            # ------- positive-pair cosines ----------------------------------
            for t in range(4):
                pit = stage.tile([128, D], F32, tag="stage")
                nc.sync.dma_start(pit[:], pi[bass.ts(t, 128), :])
                pjt = stage.tile([128, D], F32, tag="stage")
                nc.sync.dma_start(pjt[:], pj[bass.ts(t, 128), :])
                for src0, src1, acc in (
                    (pit, pit, pos_ssi),
                    (pjt, pjt, pos_ssj),
                    (pit, pjt, pos_dot),
                ):
                    snk = sink.tile([128, D], F32, tag="sink")
                    nc.vector.tensor_mul(snk[:], src0[:], src1[:])
                    nc.vector.tensor_reduce(
                        acc[:, t : t + 1], snk[:],
                        axis=mybir.AxisListType.X, op=ALU.add,
                    )
            lssi = big.tile([128, 4], F32, tag="lssi")
            lssj = big.tile([128, 4], F32, tag="lssj")
            nc.scalar.activation(lssi[:], pos_ssi[:], AF.Ln)
            nc.scalar.activation(lssj[:], pos_ssj[:], AF.Ln)
            lsum = big.tile([128, 4], F32, tag="lsum")
            nc.vector.tensor_add(lsum[:], lssi[:], lssj[:])
            rinv_ij = big.tile([128, 4], F32, tag="rinv_ij")
            nc.scalar.activation(rinv_ij[:], lsum[:], AF.Exp, scale=-0.5)
            posk = big.tile([128, 4], F32, tag="posk")
            nc.vector.tensor_mul(posk[:], pos_dot[:], rinv_ij[:])

            # ------- main: software-pipelined emission ----------------------
            # Prep of group g+1 is emitted BEFORE the matmuls of group g so
            # the g+1 sum-of-squares matmuls sit ahead of g's mains in the
            # strict PE FIFO; the dependent ln/exp/scale chain then overlaps
            # g's matmul window instead of idling the PE between groups.
            def emit_mains(jg):
                for i in range(8):
                    pt = pp_main.tile([128, 2048], F32, tag="pp_main",
                                      name=f"mm_{jg}_{i}")
                    for d in range(D_CH):
                        for jj in range(4):
                            j = jg * 4 + jj
                            nc.tensor.matmul(
                                pt[:, bass.ts(jj, 512)],
                                lhsT[d][:, bass.ts(i, 128)],
                                zT[d][:, bass.ts(j, 512)],
                                start=(d == 0), stop=(d == D_CH - 1),
                            )
                    es = esink.tile([128, 2048], BF16, tag="esink")
                    k = i * 4 + jg
                    nc.scalar.activation(
                        es[:], pt[:], AF.Exp, scale=INV_T,
                        accum_out=dacc[:, k : k + 1],
                    )

            emit_prep_group(repsT, 0, zT, 0, 4)
            emit_prep_group(repsT, 4, zT, 4, 4)
            emit_mains(0)
            emit_prep_group(repsT, 8, zT, 8, 4)
            emit_mains(1)
            emit_prep_group(repsT, 12, zT, 12, 4)
            emit_mains(2)
            emit_mains(3)

            # ------- final reduction ----------------------------------------
            dn = big.tile([128, 8], F32, tag="dn")
            nc.vector.tensor_reduce(
                dn[:], dacc[:].rearrange("p (i g) -> p i g", g=4),
                axis=mybir.AxisListType.X, op=ALU.add,
            )
            ld = big.tile([128, 8], F32, tag="ld")
            nc.scalar.activation(ld[:], dn[:], AF.Ln, bias=neg_e2[:])
            fin = big.tile([128, 2], F32, tag="fin")
            nc.vector.tensor_reduce(
                fin[:, 0:1], ld[:], axis=mybir.AxisListType.X, op=ALU.add
            )
            nc.vector.tensor_reduce(
                fin[:, 1:2], posk[:], axis=mybir.AxisListType.X, op=ALU.add
            )
            fmm = pp_main.tile([128, 2048], F32, tag="pp_main", name="fmm")
            nc.tensor.matmul(fmm[0:2, 0:1], fin[:], ones_f[:], start=True,
                             stop=True)
            outsb = big.tile([2, 1], F32, tag="outsb")
            nc.vector.tensor_copy(outsb[:], fmm[0:2, 0:1])
            nc.sync.dma_start(out_d[:], outsb[:])

    nc.compile()
    return nc


_NC_CACHE = None


def _get_program():
    global _NC_CACHE
    if _NC_CACHE is None:
        _NC_CACHE = build_program()
    return _NC_CACHE


def make_in_maps(emb_i: np.ndarray, emb_j: np.ndarray):
    emb_i = np.asarray(emb_i, dtype=np.float32)
    emb_j = np.asarray(emb_j, dtype=np.float32)
    reps = np.concatenate([emb_i, emb_j], axis=0)          # [8192, 512]
    repsT = np.ascontiguousarray(reps.T)                   # [512, 8192]
    in_maps = []
    for c in range(N_CORES):
        in_maps.append(
            {
                "repsT": repsT,
                "myT": np.ascontiguousarray(
                    repsT[:, c * ROWS_PER_CORE : (c + 1) * ROWS_PER_CORE]
                ),
                "pi": np.ascontiguousarray(
                    emb_i[c * POS_PER_CORE : (c + 1) * POS_PER_CORE]
                ),
                "pj": np.ascontiguousarray(
                    emb_j[c * POS_PER_CORE : (c + 1) * POS_PER_CORE]
                ),
            }
        )
    return in_maps


def combine_outputs(results):
    ld_sum = 0.0
    cos_sum = 0.0
    for r in results:
        o = np.asarray(r["out"], dtype=np.float64).reshape(-1)
        ld_sum += o[0]
        cos_sum += o[1]
    loss = (ld_sum - 2.0 * INV_T * cos_sum) / float(M)
    return np.float32(loss)


def kernel(emb_i: np.ndarray, emb_j: np.ndarray) -> np.ndarray:
    nc = _get_program()
    in_maps = make_in_maps(emb_i, emb_j)
    res = run_bass_kernel_spmd(nc, in_maps, list(range(N_CORES)))
    return combine_outputs(res.results)
